# revision 23
# baseline (speedup 1.0000x reference)
"""Trainium2 Bass kernel for nn_BayesianSkipgram (loss_fn).

Strategy (vocab-parallel, per sharding hint):
  - The three [V, E] tables (emb, prior_mus, prior_sigmas) are row-sharded
    across 8 NeuronCores, packed side-by-side into one [V/8, 3E] table per
    core so each context lookup is a single indirect-DMA row gather.
  - Each context position is processed by the core that owns its vocab row
    (owner-computes): the core gathers the row locally, computes its partial
    relu-sum for h and its partial hinge-loss sum. Position lists are
    padded to a multiple of 128 with masked-out dummy rows.
  - The [E] partial sum of relu rows is combined mid-kernel with an
    AllGather (1KB); the tiny MLP (mu/sigma) and the KL math replicate on
    every core. The few looked-up rows for x and the 10 negative samples are
    broadcast to all cores as side inputs.
  - Each core returns its partial likelihood + the (replicated) kl_main;
    the host sums the 8 partials.

v2 perf notes:
  - phase-A matmuls run in bf16 (fp32 PE is 1/4 rate); accumulation fp32.
  - all [1, E] row math is done in [128, 2] column form (row ops run on a
    single DVE/ACT lane).
  - phase-B elementwise math runs as single big 3D-AP ops over all T tiles.
"""
import os
import sys
import numpy as np

for _p in ("/opt/trn_rl_repo", "/root/.axon_site/_ro/trn_rl_repo"):
    if os.path.isdir(_p) and _p not in sys.path:
        sys.path.append(_p)

import concourse.bass as bass
import concourse.bacc as bacc
import concourse.mybir as mybir
import concourse.tile as tile
from concourse import bass_utils

V = 100000
E = 256
C = 8192
NEG = 10
M = 8          # cores
SH = V // M    # table rows per core
P = 128

f32 = mybir.dt.float32
bf16 = mybir.dt.bfloat16
u32 = mybir.dt.uint32
i32 = mybir.dt.int32
AF = mybir.ActivationFunctionType
ALU = mybir.AluOpType

_BUILD_CACHE: dict[int, object] = {}


def _bc3(ap, t):
    """[P, N] AP -> [P, t, N] broadcast AP (0-step middle dim)."""
    return bass.AP(ap.tensor, ap.offset, [ap.ap[0], [0, t], ap.ap[1]])


def _build(T: int, dbg: bool = False):
    """Build the SPMD program for T index tiles of 128 per core."""
    nc = bacc.Bacc("TRN2", target_bir_lowering=False, debug=False,
                   num_devices=M)

    packA = nc.dram_tensor("packA", [SH, E], f32, kind="ExternalInput").ap()
    packB = nc.dram_tensor("packB", [SH, 2 * E], f32, kind="ExternalInput").ap()
    idx = nc.dram_tensor("idx", [P, T], i32, kind="ExternalInput").ap()
    wm = nc.dram_tensor("wm", [P, T], f32, kind="ExternalInput").ap()
    wmb = nc.dram_tensor("wmb", [P, T], bf16, kind="ExternalInput").ap()
    am = nc.dram_tensor("am", [P, T], f32, kind="ExternalInput").ap()
    mwt = nc.dram_tensor("mwt", [E, E], bf16, kind="ExternalInput").ap()
    mb = nc.dram_tensor("mb", [1, E], bf16, kind="ExternalInput").ap()
    uwwt = nc.dram_tensor("uwwt", [2 * E, 2 * E], f32, kind="ExternalInput").ap()
    ubwb = nc.dram_tensor("ubwb", [1, 2 * E], f32, kind="ExternalInput").ap()
    embx = nc.dram_tensor("embx", [P, 2], bf16, kind="ExternalInput").ap()
    pmxc = nc.dram_tensor("pmxc", [P, 2], f32, kind="ExternalInput").ap()
    psxc = nc.dram_tensor("psxc", [P, 2], f32, kind="ExternalInput").ap()
    pmn = nc.dram_tensor("pmn", [NEG, E], f32, kind="ExternalInput").ap()
    psn = nc.dram_tensor("psn", [NEG, E], f32, kind="ExternalInput").ap()
    idf = nc.dram_tensor("idf", [P, P], f32, kind="ExternalInput").ap()
    idb = nc.dram_tensor("idb", [P, P], bf16, kind="ExternalInput").ap()
    out = nc.dram_tensor("out", [1, 8], f32, kind="ExternalOutput").ap()
    if dbg:
        d_sig = nc.dram_tensor("d_sig", [P, 8], f32, kind="ExternalOutput").ap()
        d_pos = nc.dram_tensor("d_pos", [P, 16], f32, kind="ExternalOutput").ap()

    with tile.TileContext(nc) as tc:
        with (
            tc.tile_pool(name="const", bufs=1) as pc,
            tc.tile_pool(name="gath", bufs=1) as pg,
            tc.tile_pool(name="work", bufs=3) as pw,
            tc.tile_pool(name="ph", bufs=1, space="PSUM") as ph,
            tc.tile_pool(name="pz", bufs=2, space="PSUM") as pz,
            tc.tile_pool(name="pt", bufs=1, space="PSUM") as pt,
            tc.tile_pool(name="pb", bufs=1, space="PSUM") as pb,
            tc.tile_pool(name="pm", bufs=2, space="PSUM") as pmp,
            tc.tile_pool(name="dram", bufs=2, space="DRAM") as dr,
        ):
            # ---------- dummy first-collective: absorbs the communicator
            # barrier + cross-core start skew concurrently with phase A.
            # Input is uninitialized internal DRAM; output value is unused.
            dcc_in = dr.tile([1, 4], f32)
            dcc_out = dr.tile([M, 4], f32)
            nc.gpsimd.collective_compute(
                "AllGather", ALU.bypass,
                replica_groups=[list(range(M))],
                ins=[dcc_in.opt()], outs=[dcc_out.opt()],
            )
            dmy2 = pc.tile([M, 4], f32)
            nc.sync.dma_start(out=dmy2[:], in_=dcc_out[:])

            # ---------- index load + gathers ----------
            idx_t = pc.tile([P, T], i32)
            nc.sync.dma_start(out=idx_t[:], in_=idx[:, :])
            ge_tiles = []
            for t in range(T):
                ge = pg.tile([P, E], bf16, tag=f"ge{t}")
                nc.gpsimd.indirect_dma_start(
                    out=ge[:], out_offset=None, in_=packA[:, :],
                    in_offset=bass.IndirectOffsetOnAxis(ap=idx_t[:, t:t + 1],
                                                        axis=0),
                )
                ge_tiles.append(ge)
            gpb = pg.tile([P, T * 2 * E], f32)
            g3 = gpb[:].rearrange("p (t c) -> p t c", t=T)
            for t in range(T):
                nc.gpsimd.indirect_dma_start(
                    out=gpb[:, t * 2 * E:(t + 1) * 2 * E], out_offset=None,
                    in_=packB[:, :],
                    in_offset=bass.IndirectOffsetOnAxis(ap=idx_t[:, t:t + 1],
                                                        axis=0),
                )

            # ---------- constant loads ----------
            wm_t = pc.tile([P, T], f32)
            nc.sync.dma_start(out=wm_t[:], in_=wm[:, :])
            wmb_t = pc.tile([P, T], bf16)
            nc.sync.dma_start(out=wmb_t[:], in_=wmb[:, :])
            am_t = pc.tile([P, T], f32)
            nc.sync.dma_start(out=am_t[:], in_=am[:, :])
            mwt_t = []
            for k in range(2):
                t_ = pc.tile([P, E], bf16, tag=f"mwt{k}")
                nc.sync.dma_start(out=t_[:], in_=mwt[k * P:(k + 1) * P, :])
                mwt_t.append(t_)
            uwwt_t = []
            for k in range(4):
                t_ = pc.tile([P, 2 * E], f32, tag=f"uwwt{k}")
                nc.sync.dma_start(out=t_[:], in_=uwwt[k * P:(k + 1) * P, :])
                uwwt_t.append(t_)
            mb_t = pc.tile([1, E], bf16)
            nc.sync.dma_start(out=mb_t[:], in_=mb[:, :])
            ubwb_t = pc.tile([1, 2 * E], f32)
            nc.sync.dma_start(out=ubwb_t[:], in_=ubwb[:, :])
            embx_t = pc.tile([P, 2], bf16)
            nc.sync.dma_start(out=embx_t[:], in_=embx[:, :])
            pmxc_t = pc.tile([P, 2], f32)
            nc.sync.dma_start(out=pmxc_t[:], in_=pmxc[:, :])
            psxc_t = pc.tile([P, 2], f32)
            nc.sync.dma_start(out=psxc_t[:], in_=psxc[:, :])
            pmn_t = pc.tile([NEG, E], f32)
            nc.sync.dma_start(out=pmn_t[:], in_=pmn[:, :])
            psn_t = pc.tile([NEG, E], f32)
            nc.sync.dma_start(out=psn_t[:], in_=psn[:, :])

            ident = pc.tile([P, P], f32)
            nc.sync.dma_start(out=ident[:], in_=idf[:, :])
            identb = pc.tile([P, P], bf16)
            nc.sync.dma_start(out=identb[:], in_=idb[:, :])
            ones_rb = pc.tile([1, P], bf16)      # bf16 ones row (bias matmuls)
            nc.vector.memset(ones_rb[:], 1.0)
            ones_row = pc.tile([1, P], f32)
            nc.vector.memset(ones_row[:], 1.0)
            ones_col = pc.tile([P, 1], f32)
            nc.vector.memset(ones_col[:], 1.0)
            sqh_row = pc.tile([1, P], f32)       # sqrt(0.5)
            nc.vector.memset(sqh_row[:], 0.7071067811865476)
            c128 = pc.tile([1, 1], f32)
            nc.vector.memset(c128[:], float(P))
            cn128 = pc.tile([1, 1], f32)
            nc.vector.memset(cn128[:], -float(P))

            # ---------- phase A: hsum += w * relu(bf16(row) @ MwT + Mb) ----
            hsum_ps = ph.tile([1, E], f32, space="PSUM")
            for t in range(T):
                gb = ge_tiles[t]
                zps = pz.tile([P, E], f32, space="PSUM", tag="z")
                for half in range(2):
                    tp = pt.tile([P, P], bf16, space="PSUM", tag="tp")
                    nc.tensor.transpose(out=tp[:],
                                        in_=gb[:, half * P:(half + 1) * P],
                                        identity=identb[:])
                    rt = pw.tile([P, P], bf16, tag="rt")
                    nc.vector.tensor_copy(out=rt[:], in_=tp[:])
                    nc.tensor.matmul(out=zps[:], lhsT=rt[:],
                                     rhs=mwt_t[half][:],
                                     start=(half == 0), stop=False)
                nc.tensor.matmul(out=zps[:], lhsT=ones_rb[0:1, :],
                                 rhs=mb_t[0:1, :], start=False, stop=True)
                relu_t = pw.tile([P, E], bf16, tag="relu")
                nc.scalar.activation(out=relu_t[:], in_=zps[:], func=AF.Relu)
                nc.tensor.matmul(out=hsum_ps[:], lhsT=wmb_t[:, t:t + 1],
                                 rhs=relu_t[:], start=(t == 0),
                                 stop=(t == T - 1), skip_group_check=True)

            # ---------- Rw in column form; ht = [h0_col | h1_col] ----------
            ht = pc.tile([P, 4], f32)
            for ec in range(2):
                rw_ps = pmp.tile([P, 1], f32, space="PSUM", tag="m1")
                for jc in range(2):
                    nc.tensor.matmul(out=rw_ps[:],
                                     lhsT=mwt_t[jc][:, ec * P:(ec + 1) * P],
                                     rhs=embx_t[:, jc:jc + 1],
                                     start=(jc == 0), stop=False)
                nc.tensor.matmul(out=rw_ps[:],
                                 lhsT=mb_t[0:1, ec * P:(ec + 1) * P],
                                 rhs=ones_rb[0:1, 0:1], start=False, stop=True)
                # h0 = relu(C * Rw)  (C * relu == relu(C*x))
                nc.scalar.activation(out=ht[:, ec:ec + 1], in_=rw_ps[:],
                                     func=AF.Relu, scale=float(C))

            # ---- mu/z matvec, h0 half (pre-CC): [1, 512] row accumulate --
            muz_ps = pb.tile([1, 2 * E], f32, space="PSUM", tag="muz")
            for kc in range(2):
                nc.tensor.matmul(out=muz_ps[:], lhsT=ht[:, kc:kc + 1],
                                 rhs=uwwt_t[kc][:], start=(kc == 0),
                                 stop=False, skip_group_check=True)

            # ---------- AllGather partial hsum ----------
            hpart = pc.tile([1, E], f32)
            nc.scalar.copy(out=hpart[:], in_=hsum_ps[:])
            cc_in = dr.tile([1, E], f32)
            cc_out = dr.tile([M, E], f32)
            nc.sync.dma_start(out=cc_in[:], in_=hpart[:])
            nc.gpsimd.collective_compute(
                "AllGather", ALU.bypass,
                replica_groups=[list(range(M))],
                ins=[cc_in.opt()], outs=[cc_out.opt()],
            )

            # ---- CC-independent work that can fill the collective window ---
            pm3 = g3[:, :, 0:E]
            ps3 = g3[:, :, E:2 * E]
            lnb = pw.tile([P, T * E], f32, tag="big1")
            lnb3 = lnb[:].rearrange("p (t c) -> p t c", t=T)
            nc.scalar.activation(out=lnb3, in_=ps3, func=AF.Ln)
            accD = pc.tile([P, T], f32)
            nc.vector.reduce_sum(out=accD[:], in_=lnb3,
                                 axis=mybir.AxisListType.X)
            nacc3 = pc.tile([NEG, 1], f32)
            n3 = pw.tile([NEG, E], f32, tag="n3")
            nc.scalar.activation(out=n3[:], in_=psn_t[:], func=AF.Ln,
                                 accum_out=nacc3[:])

            # ---------- back from CC: h1 columns ----------
            ag_t = pc.tile([M, E], f32)
            nc.sync.dma_start(out=ag_t[:], in_=cc_out[:])
            for ec in range(2):
                h1_ps = pmp.tile([P, 1], f32, space="PSUM", tag="m1")
                nc.tensor.matmul(out=h1_ps[:],
                                 lhsT=ag_t[:, ec * P:(ec + 1) * P],
                                 rhs=ones_col[0:M, 0:1], start=True, stop=True)
                nc.vector.tensor_copy(out=ht[:, 2 + ec:3 + ec], in_=h1_ps[:])

            # ---------- mu, z: finish row matvec; transpose to columns ----
            for kc in range(2, 4):
                nc.tensor.matmul(out=muz_ps[:], lhsT=ht[:, kc:kc + 1],
                                 rhs=uwwt_t[kc][:], start=False, stop=False,
                                 skip_group_check=True)
            nc.tensor.matmul(out=muz_ps[:], lhsT=ones_row[0:1, 0:1],
                             rhs=ubwb_t[0:1, :], start=False, stop=True,
                             skip_group_check=True)
            muz_row = pc.tile([1, 2 * E], f32)
            nc.scalar.copy(out=muz_row[:], in_=muz_ps[:])
            mu_col = pc.tile([P, 2], f32)
            z_col = pc.tile([P, 2], f32)
            mzt_ps = pmp.tile([P, 4], f32, space="PSUM", tag="m1")
            for q in range(4):
                nc.tensor.matmul(out=mzt_ps[:, q:q + 1],
                                 lhsT=muz_row[0:1, q * P:(q + 1) * P],
                                 rhs=ones_row[0:1, 0:1], start=True, stop=True)
            nc.vector.tensor_copy(out=mu_col[:], in_=mzt_ps[:, 0:2])
            nc.vector.tensor_copy(out=z_col[:], in_=mzt_ps[:, 2:4])

            # ---------- sigma = relu(z) + ln1p(exp(-|z|)) in columns -------
            azc = pc.tile([P, 2], f32)
            nc.vector.tensor_copy(out=azc[:], in_=z_col[:])
            azu = azc[:].bitcast(u32)
            nc.vector.tensor_scalar(out=azu, in0=azu, scalar1=0x7FFFFFFF,
                                    scalar2=None, op0=ALU.bitwise_and)
            ezc = pc.tile([P, 2], f32)
            nc.scalar.activation(out=ezc[:], in_=azc[:], func=AF.Exp,
                                 scale=-1.0)
            # ln(1+u): ACT Ln table has ~6e-13 abs error at 1.0, which breaks
            # the exact-zero underflow the reference relies on. Use
            # ln(1+u) ~= u for u < 1e-4 (exact 0 at u == 0) via select.
            l1pc = pc.tile([P, 2], f32)
            nc.scalar.activation(out=l1pc[:], in_=ezc[:], func=AF.Ln, bias=1.0)
            usm = pc.tile([P, 2], u32)
            nc.vector.tensor_scalar(out=usm[:], in0=ezc[:], scalar1=1e-4,
                                    scalar2=None, op0=ALU.is_lt)
            nc.vector.copy_predicated(out=l1pc[:], mask=usm[:], data=ezc[:])
            rzc = pc.tile([P, 2], f32)
            nc.scalar.activation(out=rzc[:], in_=z_col[:], func=AF.Relu)
            sig_col = pc.tile([P, 2], f32)
            nc.vector.tensor_tensor(out=sig_col[:], in0=l1pc[:], in1=rzc[:],
                                    op=ALU.add)
            rs_col = pc.tile([P, 2], f32)
            nc.vector.reciprocal(out=rs_col[:], in_=sig_col[:])
            lnsg_col = pc.tile([P, 2], f32)
            nc.scalar.activation(out=lnsg_col[:], in_=sig_col[:], func=AF.Ln)
            murs_col = pc.tile([P, 2], f32)
            nc.vector.tensor_tensor(out=murs_col[:], in0=mu_col[:],
                                    in1=rs_col[:], op=ALU.mult)

            # K0 = 128 - sum ln sigma
            redc = pc.tile([P, 2], f32)
            nc.vector.tensor_scalar(out=redc[:], in0=lnsg_col[:], scalar1=-1.0,
                                    scalar2=None, op0=ALU.mult)
            redc1 = pc.tile([P, 1], f32)
            nc.vector.reduce_sum(out=redc1[:], in_=redc[:],
                                 axis=mybir.AxisListType.X)
            k0_ps = pmp.tile([1, 1], f32, space="PSUM", tag="m1")
            nc.tensor.matmul(out=k0_ps[:], lhsT=redc1[:], rhs=ones_col[:],
                             start=True, stop=True)
            k0s = pc.tile([1, 1], f32)
            nc.scalar.activation(out=k0s[:], in_=k0_ps[:], func=AF.Identity,
                                 bias=c128[0:1, :])
            k0c_ps = pmp.tile([P, 1], f32, space="PSUM", tag="m1")
            nc.tensor.matmul(out=k0c_ps[:], lhsT=ones_row[0:1, :],
                             rhs=k0s[0:1, :], start=True, stop=True)
            k0col = pc.tile([P, 1], f32)
            nc.vector.tensor_copy(out=k0col[:], in_=k0c_ps[:])

            # ---------- rows + broadcasts: SA_b, SAMu_b ----------
            rsr_ps = pmp.tile([1, E], f32, space="PSUM", tag="m1")
            for ec in range(2):
                nc.tensor.matmul(out=rsr_ps[0:1, ec * P:(ec + 1) * P],
                                 lhsT=rs_col[:, ec:ec + 1], rhs=ident[:],
                                 start=True, stop=True)
            rs_row = pc.tile([1, E], f32)
            nc.vector.tensor_copy(out=rs_row[:], in_=rsr_ps[:])
            mrr_ps = pmp.tile([1, E], f32, space="PSUM", tag="m1")
            for ec in range(2):
                nc.tensor.matmul(out=mrr_ps[0:1, ec * P:(ec + 1) * P],
                                 lhsT=murs_col[:, ec:ec + 1], rhs=ident[:],
                                 start=True, stop=True)
            murs_row = pc.tile([1, E], f32)
            nc.vector.tensor_copy(out=murs_row[:], in_=mrr_ps[:])

            sab_ps = pb.tile([P, E], f32, space="PSUM", tag="bc")
            nc.tensor.matmul(out=sab_ps[:], lhsT=sqh_row[0:1, :],
                             rhs=rs_row[0:1, :], start=True, stop=True)
            SA_b = pc.tile([P, E], f32)
            nc.vector.tensor_copy(out=SA_b[:], in_=sab_ps[:])
            smu_ps = pb.tile([P, E], f32, space="PSUM", tag="bc")
            nc.tensor.matmul(out=smu_ps[:], lhsT=sqh_row[0:1, :],
                             rhs=murs_row[0:1, :], start=True, stop=True)
            SAMu_b = pc.tile([P, E], f32)
            nc.vector.tensor_copy(out=SAMu_b[:], in_=smu_ps[:])

            # ---------- phase B (bf16, chunked over tile groups) ----------
            # pos = K0' + sum_e (sa*(|pm| - mu))^2 - sum_e (sa*ps)^2
            #       + sum_e ln ps
            accA = pc.tile([P, T], f32)
            accC = pc.tile([P, T], f32)
            CHG = 3
            for c0 in range(0, T, CHG):
                g = min(CHG, T - c0)
                SA3 = _bc3(SA_b[:], g)
                SAMu3 = _bc3(SAMu_b[:], g)
                pmc = g3[:, c0:c0 + g, 0:E]
                psc = g3[:, c0:c0 + g, E:2 * E]
                tb = pw.tile([P, CHG * E], f32, tag="chA")
                tb3 = tb[:, 0:g * E].rearrange("p (t c) -> p t c", t=g)
                nc.vector.tensor_tensor(out=tb3, in0=pmc, in1=SA3,
                                        op=ALU.mult)
                nc.scalar.activation(out=tb[:, 0:g * E], in_=tb[:, 0:g * E],
                                     func=AF.Abs)
                nc.vector.tensor_tensor(out=tb3, in0=tb3, in1=SAMu3,
                                        op=ALU.subtract)
                nc.scalar.activation(out=tb[:, 0:g * E], in_=tb[:, 0:g * E],
                                     func=AF.Square)
                nc.vector.reduce_sum(out=accA[:, c0:c0 + g], in_=tb3,
                                     axis=mybir.AxisListType.X)
                wbt = pw.tile([P, CHG * E], f32, tag="chB")
                wb3 = wbt[:, 0:g * E].rearrange("p (t c) -> p t c", t=g)
                nc.vector.tensor_tensor(out=wb3, in0=psc, in1=SA3,
                                        op=ALU.mult)
                nc.scalar.activation(out=wbt[:, 0:g * E], in_=wbt[:, 0:g * E],
                                     func=AF.Square)
                nc.vector.reduce_sum(out=accC[:, c0:c0 + g], in_=wb3,
                                     axis=mybir.AxisListType.X)
            pos_all = pc.tile([P, T], f32)
            nc.vector.tensor_tensor(out=pos_all[:], in0=accA[:], in1=accC[:],
                                    op=ALU.subtract)
            nc.vector.tensor_tensor(out=pos_all[:], in0=pos_all[:],
                                    in1=accD[:], op=ALU.add)
            k0b = _bc3(k0col[:], T)  # [P, T, 1] broadcast
            posv = pos_all[:].rearrange("p (t o) -> p t o", t=T)
            nc.vector.tensor_tensor(out=posv, in0=posv, in1=k0b, op=ALU.add)
            nc.vector.tensor_tensor(out=pos_all[:], in0=pos_all[:],
                                    in1=wm_t[:], op=ALU.mult)
            nc.vector.tensor_tensor(out=pos_all[:], in0=pos_all[:],
                                    in1=am_t[:], op=ALU.add)

            # ---------- neg_kl rows ----------
            nacc1 = pc.tile([NEG, 1], f32)
            nacc2 = pc.tile([NEG, 1], f32)
            n1 = pc.tile([NEG, E], f32)
            nc.vector.tensor_tensor(out=n1[:], in0=pmn_t[:], in1=SA_b[0:NEG, :],
                                    op=ALU.mult)
            nc.scalar.activation(out=n1[:], in_=n1[:], func=AF.Abs)
            nc.vector.tensor_tensor(out=n1[:], in0=n1[:], in1=SAMu_b[0:NEG, :],
                                    op=ALU.subtract)
            nc.scalar.activation(out=n1[:], in_=n1[:], func=AF.Square,
                                 accum_out=nacc1[:])
            n2 = pc.tile([NEG, E], f32)
            nc.vector.tensor_tensor(out=n2[:], in0=psn_t[:], in1=SA_b[0:NEG, :],
                                    op=ALU.mult)
            nc.scalar.activation(out=n2[:], in_=n2[:], func=AF.Square,
                                 accum_out=nacc2[:])
            nk = pc.tile([NEG, 1], f32)
            nc.vector.tensor_tensor(out=nk[:], in0=nacc1[:], in1=nacc2[:],
                                    op=ALU.subtract)
            nc.vector.tensor_tensor(out=nk[:], in0=nk[:], in1=nacc3[:],
                                    op=ALU.add)
            nc.vector.tensor_tensor(out=nk[:], in0=nk[:], in1=k0col[0:NEG, :],
                                    op=ALU.add)
            bneg = pc.tile([NEG, 1], f32)
            nc.vector.tensor_scalar(out=bneg[:], in0=nk[:], scalar1=-1.0,
                                    scalar2=1.0, op0=ALU.mult, op1=ALU.add)
            b_ps = pmp.tile([1, NEG], f32, space="PSUM", tag="m1")
            nc.tensor.transpose(out=b_ps[:], in_=bneg[:],
                                identity=ident[0:NEG, 0:NEG])
            b_row = pc.tile([1, NEG], f32)
            nc.vector.tensor_copy(out=b_row[:], in_=b_ps[:])
            bb_ps = pb.tile([P, NEG], f32, space="PSUM", tag="bc")
            nc.tensor.matmul(out=bb_ps[:], lhsT=ones_row[0:1, :],
                             rhs=b_row[0:1, :], start=True, stop=True)
            B_b = pc.tile([P, NEG], f32)
            nc.vector.tensor_copy(out=B_b[:], in_=bb_ps[:])

            # ---------- hinge: relu(pos_c + b_n) over all (n, c) pairs ---
            hb = pc.tile([P, NEG * T], f32)
            hb3 = hb[:].rearrange("p (n t) -> p n t", n=NEG)
            pos_ap = pos_all[:]
            pos_bc = bass.AP(pos_ap.tensor, pos_ap.offset,
                             [pos_ap.ap[0], [0, NEG], pos_ap.ap[1]])
            bb_ap = B_b[:]
            bb_bc = bass.AP(bb_ap.tensor, bb_ap.offset,
                            [bb_ap.ap[0], bb_ap.ap[1], [0, T]])
            nc.vector.tensor_tensor(out=hb3, in0=pos_bc, in1=bb_bc,
                                    op=ALU.add)
            nc.scalar.activation(out=hb[:], in_=hb[:], func=AF.Relu)
            hsumcol = pc.tile([P, 1], f32)
            nc.vector.reduce_sum(out=hsumcol[:], in_=hb3,
                                 axis=mybir.AxisListType.XY)
            lik_ps = pmp.tile([1, 1], f32, space="PSUM", tag="m1")
            nc.tensor.matmul(out=lik_ps[:], lhsT=hsumcol[:], rhs=ones_col[:],
                             start=True, stop=True)
            lik = pc.tile([1, 1], f32)
            nc.scalar.copy(out=lik[:], in_=lik_ps[:])

            # ---------- kl_main in columns ----------
            # 0.5*( sum_e [2 ln psx - ln sig + sig/varp + (pmx-mu)^2/varp]
            #       - E )
            sqp = pc.tile([P, 2], f32)
            nc.scalar.activation(out=sqp[:], in_=psxc_t[:], func=AF.Square)
            rvp = pc.tile([P, 2], f32)
            nc.vector.reciprocal(out=rvp[:], in_=sqp[:])
            kt = pc.tile([P, 2], f32)
            nc.scalar.activation(out=kt[:], in_=psxc_t[:], func=AF.Ln)
            nc.vector.tensor_scalar(out=kt[:], in0=kt[:], scalar1=2.0,
                                    scalar2=None, op0=ALU.mult)
            nc.vector.tensor_tensor(out=kt[:], in0=kt[:], in1=lnsg_col[:],
                                    op=ALU.subtract)
            t1c = pc.tile([P, 2], f32)
            nc.vector.tensor_tensor(out=t1c[:], in0=sig_col[:], in1=rvp[:],
                                    op=ALU.mult)
            nc.vector.tensor_tensor(out=kt[:], in0=kt[:], in1=t1c[:],
                                    op=ALU.add)
            dc = pc.tile([P, 2], f32)
            nc.vector.tensor_tensor(out=dc[:], in0=pmxc_t[:], in1=mu_col[:],
                                    op=ALU.subtract)
            nc.vector.tensor_tensor(out=dc[:], in0=dc[:], in1=dc[:],
                                    op=ALU.mult)
            nc.vector.tensor_tensor(out=dc[:], in0=dc[:], in1=rvp[:],
                                    op=ALU.mult)
            nc.vector.tensor_tensor(out=kt[:], in0=kt[:], in1=dc[:],
                                    op=ALU.add)
            ktr = pc.tile([P, 1], f32)
            nc.vector.reduce_sum(out=ktr[:], in_=kt[:],
                                 axis=mybir.AxisListType.X)
            kl_ps = pmp.tile([1, 1], f32, space="PSUM", tag="m1")
            nc.tensor.matmul(out=kl_ps[:], lhsT=ktr[:], rhs=ones_col[:],
                             start=True, stop=True)
            klm = pc.tile([1, 1], f32)
            nc.scalar.activation(out=klm[:], in_=kl_ps[:], func=AF.Identity,
                                 scale=0.5, bias=cn128[0:1, :])

            # ---------- output row ----------
            orow = pc.tile([1, 8], f32)
            nc.vector.memset(orow[:], 0.0)
            nc.vector.tensor_tensor(out=orow[:, 0:1], in0=lik[:], in1=klm[:],
                                    op=ALU.add)
            nc.vector.tensor_copy(out=orow[:, 1:2], in_=klm[:])
            nc.vector.tensor_copy(out=orow[:, 2:3], in_=lik[:])
            hdr = pc.tile([P, 1], f32)
            nc.vector.reduce_sum(out=hdr[:], in_=ht[:, 2:4],
                                 axis=mybir.AxisListType.X)
            hd_ps = pmp.tile([1, 1], f32, space="PSUM", tag="m1")
            nc.tensor.matmul(out=hd_ps[:], lhsT=hdr[:], rhs=ones_col[:],
                             start=True, stop=True)
            nc.vector.tensor_copy(out=orow[:, 3:4], in_=hd_ps[:])
            sdr = pc.tile([P, 1], f32)
            nc.vector.reduce_sum(out=sdr[:], in_=sig_col[:],
                                 axis=mybir.AxisListType.X)
            sd_ps = pmp.tile([1, 1], f32, space="PSUM", tag="m1")
            nc.tensor.matmul(out=sd_ps[:], lhsT=sdr[:], rhs=ones_col[:],
                             start=True, stop=True)
            nc.vector.tensor_copy(out=orow[:, 4:5], in_=sd_ps[:])
            mdr = pc.tile([P, 1], f32)
            nc.vector.reduce_sum(out=mdr[:], in_=mu_col[:],
                                 axis=mybir.AxisListType.X)
            md_ps = pmp.tile([1, 1], f32, space="PSUM", tag="m1")
            nc.tensor.matmul(out=md_ps[:], lhsT=mdr[:], rhs=ones_col[:],
                             start=True, stop=True)
            nc.vector.tensor_copy(out=orow[:, 5:6], in_=md_ps[:])
            nc.sync.dma_start(out=out[:, :], in_=orow[:])
            if dbg:
                nc.sync.dma_start(out=d_sig[:, 0:2], in_=sig_col[:])
                nc.sync.dma_start(out=d_sig[:, 2:4], in_=rs_col[:])
                nc.sync.dma_start(out=d_sig[:, 4:6], in_=mu_col[:])
                nc.sync.dma_start(out=d_sig[:, 6:8], in_=z_col[:])
                nc.sync.dma_start(out=d_pos[:, 0:T], in_=pos_all[:])
    nc.compile()
    return nc


def _prep_inputs(x, context, negative_samples, emb, M_w, M_b, U_w, U_b,
                 W_w, W_b, prior_mus, prior_sigmas):
    x = np.asarray(x).reshape(-1)
    ctx = np.asarray(context).reshape(-1).astype(np.int64)
    neg = np.asarray(negative_samples).reshape(-1).astype(np.int64)
    emb = np.ascontiguousarray(np.asarray(emb, dtype=np.float32))
    prior_mus = np.ascontiguousarray(np.asarray(prior_mus, dtype=np.float32))
    prior_sigmas = np.ascontiguousarray(np.asarray(prior_sigmas,
                                                   dtype=np.float32))

    owner = ctx // SH
    local = (ctx % SH).astype(np.int32)
    counts = np.bincount(owner, minlength=M)
    T = max(1, int(np.ceil(counts.max() / P)))

    packB = np.concatenate([prior_mus, prior_sigmas], axis=1)  # [V, 512]

    import ml_dtypes
    bfl = ml_dtypes.bfloat16
    mwt = np.ascontiguousarray(np.asarray(M_w, np.float32).T.astype(bfl))
    uwwt = np.ascontiguousarray(np.concatenate(
        [np.asarray(U_w, np.float32).T, np.asarray(W_w, np.float32).T],
        axis=1))  # [512, 512] = [U^T | W^T]
    mb = np.asarray(M_b, np.float32).reshape(1, E).astype(bfl)
    ubwb = np.concatenate([np.asarray(U_b, np.float32).reshape(1, E),
                           np.asarray(W_b, np.float32).reshape(1, E)], axis=1)
    xi = int(x[0])
    embx = np.ascontiguousarray(emb[xi].reshape(2, P).T.astype(bfl))
    pmxc = np.ascontiguousarray(prior_mus[xi].reshape(2, P).T)
    psxc = np.ascontiguousarray(prior_sigmas[xi].reshape(2, P).T)
    pmn = np.ascontiguousarray(prior_mus[neg])
    psn = np.ascontiguousarray(prior_sigmas[neg])
    idf = np.eye(P, dtype=np.float32)
    idb = np.eye(P, dtype=np.float32).astype(bfl)

    in_maps = []
    for k in range(M):
        sel = np.nonzero(owner == k)[0]
        nk_ = len(sel)
        idxk = np.zeros(T * P, np.int32)
        idxk[:nk_] = local[sel]
        wmk = np.zeros(T * P, np.float32)
        wmk[:nk_] = 1.0
        amk = np.full(T * P, -1e30, np.float32)
        amk[:nk_] = 0.0
        wm_pt = np.ascontiguousarray(wmk.reshape(T, P).T)
        in_maps.append({
            "packA": emb[k * SH:(k + 1) * SH],
            "packB": packB[k * SH:(k + 1) * SH],
            "idx": np.ascontiguousarray(idxk.reshape(T, P).T),
            "wm": wm_pt,
            "wmb": wm_pt.astype(bfl),
            "am": np.ascontiguousarray(amk.reshape(T, P).T),
            "mwt": mwt, "mb": mb, "uwwt": uwwt, "ubwb": ubwb,
            "embx": embx, "pmxc": pmxc, "psxc": psxc, "pmn": pmn, "psn": psn,
            "idf": idf, "idb": idb,
        })
    return T, in_maps


def run_on_device(T, in_maps, trace=False):
    nc = _BUILD_CACHE.get(T)
    if nc is None:
        nc = _build(T)
        _BUILD_CACHE[T] = nc
    return bass_utils.run_bass_kernel_spmd(
        nc, in_maps, core_ids=list(range(M)), trace=trace)


def kernel(**inputs) -> np.ndarray:
    T, in_maps = _prep_inputs(**inputs)
    res = run_on_device(T, in_maps)
    outs = [res.results[k]["out"][0] for k in range(M)]
    lik = np.float32(0.0)
    for k in range(M):
        lik = np.float32(lik + np.float32(outs[k][2]))
    total = np.float32(np.float32(outs[0][1]) + lik)
    return np.array([total], dtype=np.float32)


if __name__ == "__main__":
    d = np.load("/tmp/ref_inputs.npz")
    inp = {k: d[k] for k in d.files}
    out = kernel(**inp)
    print("kernel output:", out)


# revision 24
# speedup vs baseline: 1.1818x; 1.1818x over previous
"""Trainium2 Bass kernel for nn_BayesianSkipgram (loss_fn).

Strategy (vocab-parallel, per sharding hint):
  - The three [V, E] tables (emb, prior_mus, prior_sigmas) are row-sharded
    across 8 NeuronCores, packed side-by-side into one [V/8, 3E] table per
    core so each context lookup is a single indirect-DMA row gather.
  - Each context position is processed by the core that owns its vocab row
    (owner-computes): the core gathers the row locally, computes its partial
    relu-sum for h and its partial hinge-loss sum. Position lists are
    padded to a multiple of 128 with masked-out dummy rows.
  - The [E] partial sum of relu rows is combined mid-kernel with an
    AllGather (1KB); the tiny MLP (mu/sigma) and the KL math replicate on
    every core. The few looked-up rows for x and the 10 negative samples are
    broadcast to all cores as side inputs.
  - Each core returns its partial likelihood + the (replicated) kl_main;
    the host sums the 8 partials.

v2 perf notes:
  - phase-A matmuls run in bf16 (fp32 PE is 1/4 rate); accumulation fp32.
  - all [1, E] row math is done in [128, 2] column form (row ops run on a
    single DVE/ACT lane).
  - phase-B elementwise math runs as single big 3D-AP ops over all T tiles.
"""
import os
import sys
import numpy as np

for _p in ("/opt/trn_rl_repo", "/root/.axon_site/_ro/trn_rl_repo"):
    if os.path.isdir(_p) and _p not in sys.path:
        sys.path.append(_p)

import concourse.bass as bass
import concourse.bacc as bacc
import concourse.mybir as mybir
import concourse.tile as tile
from concourse import bass_utils

V = 100000
E = 256
C = 8192
NEG = 10
M = 8          # cores
SH = V // M    # table rows per core
P = 128

f32 = mybir.dt.float32
bf16 = mybir.dt.bfloat16
u32 = mybir.dt.uint32
i32 = mybir.dt.int32
AF = mybir.ActivationFunctionType
ALU = mybir.AluOpType

_BUILD_CACHE: dict[int, object] = {}


def _bc3(ap, t):
    """[P, N] AP -> [P, t, N] broadcast AP (0-step middle dim)."""
    return bass.AP(ap.tensor, ap.offset, [ap.ap[0], [0, t], ap.ap[1]])


def _build(T: int, dbg: bool = False):
    """Build the SPMD program for T index tiles of 128 per core."""
    nc = bacc.Bacc("TRN2", target_bir_lowering=False, debug=False,
                   num_devices=M)

    packA = nc.dram_tensor("packA", [SH, E], f32, kind="ExternalInput").ap()
    packB = nc.dram_tensor("packB", [SH, 2 * E], f32, kind="ExternalInput").ap()
    idx = nc.dram_tensor("idx", [P, T], i32, kind="ExternalInput").ap()
    wm = nc.dram_tensor("wm", [P, T], f32, kind="ExternalInput").ap()
    wmb = nc.dram_tensor("wmb", [P, T], bf16, kind="ExternalInput").ap()
    am = nc.dram_tensor("am", [P, T], f32, kind="ExternalInput").ap()
    mwt = nc.dram_tensor("mwt", [E, E], bf16, kind="ExternalInput").ap()
    mb = nc.dram_tensor("mb", [1, E], bf16, kind="ExternalInput").ap()
    uwwt = nc.dram_tensor("uwwt", [2 * E, 2 * E], f32, kind="ExternalInput").ap()
    ubwb = nc.dram_tensor("ubwb", [1, 2 * E], f32, kind="ExternalInput").ap()
    embx = nc.dram_tensor("embx", [P, 2], bf16, kind="ExternalInput").ap()
    pmxc = nc.dram_tensor("pmxc", [P, 2], f32, kind="ExternalInput").ap()
    psxc = nc.dram_tensor("psxc", [P, 2], f32, kind="ExternalInput").ap()
    pmn = nc.dram_tensor("pmn", [NEG, E], f32, kind="ExternalInput").ap()
    psn = nc.dram_tensor("psn", [NEG, E], f32, kind="ExternalInput").ap()
    idf = nc.dram_tensor("idf", [P, P], f32, kind="ExternalInput").ap()
    idb = nc.dram_tensor("idb", [P, P], bf16, kind="ExternalInput").ap()
    out = nc.dram_tensor("out", [1, 8], f32, kind="ExternalOutput").ap()
    if dbg:
        d_sig = nc.dram_tensor("d_sig", [P, 8], f32, kind="ExternalOutput").ap()
        d_pos = nc.dram_tensor("d_pos", [P, 16], f32, kind="ExternalOutput").ap()

    with tile.TileContext(nc) as tc:
        with (
            tc.tile_pool(name="const", bufs=1) as pc,
            tc.tile_pool(name="gath", bufs=1) as pg,
            tc.tile_pool(name="work", bufs=3) as pw,
            tc.tile_pool(name="ph", bufs=1, space="PSUM") as ph,
            tc.tile_pool(name="pz", bufs=2, space="PSUM") as pz,
            tc.tile_pool(name="pt", bufs=1, space="PSUM") as pt,
            tc.tile_pool(name="pb", bufs=1, space="PSUM") as pb,
            tc.tile_pool(name="pm", bufs=2, space="PSUM") as pmp,
            tc.tile_pool(name="dram", bufs=2, space="DRAM") as dr,
        ):
            # ---------- index load + gathers ----------
            idx_t = pc.tile([P, T], i32)
            nc.sync.dma_start(out=idx_t[:], in_=idx[:, :])
            ge_tiles = []
            for t in range(T):
                ge = pg.tile([P, E], bf16, tag=f"ge{t}")
                nc.gpsimd.indirect_dma_start(
                    out=ge[:], out_offset=None, in_=packA[:, :],
                    in_offset=bass.IndirectOffsetOnAxis(ap=idx_t[:, t:t + 1],
                                                        axis=0),
                )
                ge_tiles.append(ge)
            gpb = pg.tile([P, T * 2 * E], f32)
            g3 = gpb[:].rearrange("p (t c) -> p t c", t=T)
            for t in range(T):
                nc.gpsimd.indirect_dma_start(
                    out=gpb[:, t * 2 * E:(t + 1) * 2 * E], out_offset=None,
                    in_=packB[:, :],
                    in_offset=bass.IndirectOffsetOnAxis(ap=idx_t[:, t:t + 1],
                                                        axis=0),
                )

            # ---------- constant loads ----------
            wm_t = pc.tile([P, T], f32)
            nc.sync.dma_start(out=wm_t[:], in_=wm[:, :])
            wmb_t = pc.tile([P, T], bf16)
            nc.sync.dma_start(out=wmb_t[:], in_=wmb[:, :])
            am_t = pc.tile([P, T], f32)
            nc.sync.dma_start(out=am_t[:], in_=am[:, :])
            mwt_t = []
            for k in range(2):
                t_ = pc.tile([P, E], bf16, tag=f"mwt{k}")
                nc.sync.dma_start(out=t_[:], in_=mwt[k * P:(k + 1) * P, :])
                mwt_t.append(t_)
            uwwt_t = []
            for k in range(4):
                t_ = pc.tile([P, 2 * E], f32, tag=f"uwwt{k}")
                nc.sync.dma_start(out=t_[:], in_=uwwt[k * P:(k + 1) * P, :])
                uwwt_t.append(t_)
            mb_t = pc.tile([1, E], bf16)
            nc.sync.dma_start(out=mb_t[:], in_=mb[:, :])
            ubwb_t = pc.tile([1, 2 * E], f32)
            nc.sync.dma_start(out=ubwb_t[:], in_=ubwb[:, :])
            embx_t = pc.tile([P, 2], bf16)
            nc.sync.dma_start(out=embx_t[:], in_=embx[:, :])
            pmxc_t = pc.tile([P, 2], f32)
            nc.sync.dma_start(out=pmxc_t[:], in_=pmxc[:, :])
            psxc_t = pc.tile([P, 2], f32)
            nc.sync.dma_start(out=psxc_t[:], in_=psxc[:, :])
            pmn_t = pc.tile([NEG, E], f32)
            nc.sync.dma_start(out=pmn_t[:], in_=pmn[:, :])
            psn_t = pc.tile([NEG, E], f32)
            nc.sync.dma_start(out=psn_t[:], in_=psn[:, :])

            ident = pc.tile([P, P], f32)
            nc.sync.dma_start(out=ident[:], in_=idf[:, :])
            identb = pc.tile([P, P], bf16)
            nc.sync.dma_start(out=identb[:], in_=idb[:, :])
            ones_rb = pc.tile([1, P], bf16)      # bf16 ones row (bias matmuls)
            nc.vector.memset(ones_rb[:], 1.0)
            ones_row = pc.tile([1, P], f32)
            nc.vector.memset(ones_row[:], 1.0)
            ones_col = pc.tile([P, 1], f32)
            nc.vector.memset(ones_col[:], 1.0)
            sqh_row = pc.tile([1, P], f32)       # sqrt(0.5)
            nc.vector.memset(sqh_row[:], 0.7071067811865476)
            c128 = pc.tile([1, 1], f32)
            nc.vector.memset(c128[:], float(P))
            cn128 = pc.tile([1, 1], f32)
            nc.vector.memset(cn128[:], -float(P))

            # ---------- phase A: hsum += w * relu(bf16(row) @ MwT + Mb) ----
            hsum_ps = ph.tile([1, E], f32, space="PSUM")
            for t in range(T):
                gb = ge_tiles[t]
                zps = pz.tile([P, E], f32, space="PSUM", tag="z")
                for half in range(2):
                    tp = pt.tile([P, P], bf16, space="PSUM", tag="tp")
                    nc.tensor.transpose(out=tp[:],
                                        in_=gb[:, half * P:(half + 1) * P],
                                        identity=identb[:])
                    rt = pw.tile([P, P], bf16, tag="rt")
                    nc.vector.tensor_copy(out=rt[:], in_=tp[:])
                    nc.tensor.matmul(out=zps[:], lhsT=rt[:],
                                     rhs=mwt_t[half][:],
                                     start=(half == 0), stop=False)
                nc.tensor.matmul(out=zps[:], lhsT=ones_rb[0:1, :],
                                 rhs=mb_t[0:1, :], start=False, stop=True)
                relu_t = pw.tile([P, E], bf16, tag="relu")
                nc.scalar.activation(out=relu_t[:], in_=zps[:], func=AF.Relu)
                nc.tensor.matmul(out=hsum_ps[:], lhsT=wmb_t[:, t:t + 1],
                                 rhs=relu_t[:], start=(t == 0),
                                 stop=(t == T - 1), skip_group_check=True)

            # ---------- Rw in column form; ht = [h0_col | h1_col] ----------
            ht = pc.tile([P, 4], f32)
            for ec in range(2):
                rw_ps = pmp.tile([P, 1], f32, space="PSUM", tag="m1")
                for jc in range(2):
                    nc.tensor.matmul(out=rw_ps[:],
                                     lhsT=mwt_t[jc][:, ec * P:(ec + 1) * P],
                                     rhs=embx_t[:, jc:jc + 1],
                                     start=(jc == 0), stop=False)
                nc.tensor.matmul(out=rw_ps[:],
                                 lhsT=mb_t[0:1, ec * P:(ec + 1) * P],
                                 rhs=ones_rb[0:1, 0:1], start=False, stop=True)
                # h0 = relu(C * Rw)  (C * relu == relu(C*x))
                nc.scalar.activation(out=ht[:, ec:ec + 1], in_=rw_ps[:],
                                     func=AF.Relu, scale=float(C))

            # ---- mu/z matvec, h0 half (pre-CC): [1, 512] row accumulate --
            muz_ps = pb.tile([1, 2 * E], f32, space="PSUM", tag="muz")
            for kc in range(2):
                nc.tensor.matmul(out=muz_ps[:], lhsT=ht[:, kc:kc + 1],
                                 rhs=uwwt_t[kc][:], start=(kc == 0),
                                 stop=False, skip_group_check=True)

            # ---------- AllGather partial hsum ----------
            hpart = pc.tile([1, E], f32)
            nc.scalar.copy(out=hpart[:], in_=hsum_ps[:])
            cc_in = dr.tile([1, E], f32)
            cc_out = dr.tile([M, E], f32)
            nc.sync.dma_start(out=cc_in[:], in_=hpart[:])
            nc.gpsimd.collective_compute(
                "AllGather", ALU.bypass,
                replica_groups=[list(range(M))],
                ins=[cc_in.opt()], outs=[cc_out.opt()],
            )

            # ---- CC-independent work that can fill the collective window ---
            pm3 = g3[:, :, 0:E]
            ps3 = g3[:, :, E:2 * E]
            lnb = pw.tile([P, T * E], f32, tag="big1")
            lnb3 = lnb[:].rearrange("p (t c) -> p t c", t=T)
            nc.scalar.activation(out=lnb3, in_=ps3, func=AF.Ln)
            accD = pc.tile([P, T], f32)
            nc.vector.reduce_sum(out=accD[:], in_=lnb3,
                                 axis=mybir.AxisListType.X)
            nacc3 = pc.tile([NEG, 1], f32)
            n3 = pw.tile([NEG, E], f32, tag="n3")
            nc.scalar.activation(out=n3[:], in_=psn_t[:], func=AF.Ln,
                                 accum_out=nacc3[:])

            # ---------- back from CC: h1 columns ----------
            ag_t = pc.tile([M, E], f32)
            nc.sync.dma_start(out=ag_t[:], in_=cc_out[:])
            for ec in range(2):
                h1_ps = pmp.tile([P, 1], f32, space="PSUM", tag="m1")
                nc.tensor.matmul(out=h1_ps[:],
                                 lhsT=ag_t[:, ec * P:(ec + 1) * P],
                                 rhs=ones_col[0:M, 0:1], start=True, stop=True)
                nc.vector.tensor_copy(out=ht[:, 2 + ec:3 + ec], in_=h1_ps[:])

            # ---------- mu, z: finish row matvec; transpose to columns ----
            for kc in range(2, 4):
                nc.tensor.matmul(out=muz_ps[:], lhsT=ht[:, kc:kc + 1],
                                 rhs=uwwt_t[kc][:], start=False, stop=False,
                                 skip_group_check=True)
            nc.tensor.matmul(out=muz_ps[:], lhsT=ones_row[0:1, 0:1],
                             rhs=ubwb_t[0:1, :], start=False, stop=True,
                             skip_group_check=True)
            muz_row = pc.tile([1, 2 * E], f32)
            nc.scalar.copy(out=muz_row[:], in_=muz_ps[:])
            mu_col = pc.tile([P, 2], f32)
            z_col = pc.tile([P, 2], f32)
            mzt_ps = pmp.tile([P, 4], f32, space="PSUM", tag="m1")
            for q in range(4):
                nc.tensor.matmul(out=mzt_ps[:, q:q + 1],
                                 lhsT=muz_row[0:1, q * P:(q + 1) * P],
                                 rhs=ones_row[0:1, 0:1], start=True, stop=True)
            nc.vector.tensor_copy(out=mu_col[:], in_=mzt_ps[:, 0:2])
            nc.vector.tensor_copy(out=z_col[:], in_=mzt_ps[:, 2:4])

            # ---------- sigma = relu(z) + ln1p(exp(-|z|)) in columns -------
            azc = pc.tile([P, 2], f32)
            nc.vector.tensor_copy(out=azc[:], in_=z_col[:])
            azu = azc[:].bitcast(u32)
            nc.vector.tensor_scalar(out=azu, in0=azu, scalar1=0x7FFFFFFF,
                                    scalar2=None, op0=ALU.bitwise_and)
            ezc = pc.tile([P, 2], f32)
            nc.scalar.activation(out=ezc[:], in_=azc[:], func=AF.Exp,
                                 scale=-1.0)
            # ln(1+u): ACT Ln table has ~6e-13 abs error at 1.0, which breaks
            # the exact-zero underflow the reference relies on. Use
            # ln(1+u) ~= u for u < 1e-4 (exact 0 at u == 0) via select.
            l1pc = pc.tile([P, 2], f32)
            nc.scalar.activation(out=l1pc[:], in_=ezc[:], func=AF.Ln, bias=1.0)
            usm = pc.tile([P, 2], u32)
            nc.vector.tensor_scalar(out=usm[:], in0=ezc[:], scalar1=1e-4,
                                    scalar2=None, op0=ALU.is_lt)
            nc.vector.copy_predicated(out=l1pc[:], mask=usm[:], data=ezc[:])
            rzc = pc.tile([P, 2], f32)
            nc.scalar.activation(out=rzc[:], in_=z_col[:], func=AF.Relu)
            sig_col = pc.tile([P, 2], f32)
            nc.vector.tensor_tensor(out=sig_col[:], in0=l1pc[:], in1=rzc[:],
                                    op=ALU.add)
            rs_col = pc.tile([P, 2], f32)
            nc.vector.reciprocal(out=rs_col[:], in_=sig_col[:])
            lnsg_col = pc.tile([P, 2], f32)
            nc.scalar.activation(out=lnsg_col[:], in_=sig_col[:], func=AF.Ln)
            murs_col = pc.tile([P, 2], f32)
            nc.vector.tensor_tensor(out=murs_col[:], in0=mu_col[:],
                                    in1=rs_col[:], op=ALU.mult)

            # K0 = 128 - sum ln sigma
            redc = pc.tile([P, 2], f32)
            nc.vector.tensor_scalar(out=redc[:], in0=lnsg_col[:], scalar1=-1.0,
                                    scalar2=None, op0=ALU.mult)
            redc1 = pc.tile([P, 1], f32)
            nc.vector.reduce_sum(out=redc1[:], in_=redc[:],
                                 axis=mybir.AxisListType.X)
            k0_ps = pmp.tile([1, 1], f32, space="PSUM", tag="m1")
            nc.tensor.matmul(out=k0_ps[:], lhsT=redc1[:], rhs=ones_col[:],
                             start=True, stop=True)
            k0s = pc.tile([1, 1], f32)
            nc.scalar.activation(out=k0s[:], in_=k0_ps[:], func=AF.Identity,
                                 bias=c128[0:1, :])
            k0c_ps = pmp.tile([P, 1], f32, space="PSUM", tag="m1")
            nc.tensor.matmul(out=k0c_ps[:], lhsT=ones_row[0:1, :],
                             rhs=k0s[0:1, :], start=True, stop=True)
            k0col = pc.tile([P, 1], f32)
            nc.vector.tensor_copy(out=k0col[:], in_=k0c_ps[:])

            # ---------- rows + broadcasts: SA_b, SAMu_b ----------
            rsr_ps = pmp.tile([1, E], f32, space="PSUM", tag="m1")
            for ec in range(2):
                nc.tensor.matmul(out=rsr_ps[0:1, ec * P:(ec + 1) * P],
                                 lhsT=rs_col[:, ec:ec + 1], rhs=ident[:],
                                 start=True, stop=True)
            rs_row = pc.tile([1, E], f32)
            nc.vector.tensor_copy(out=rs_row[:], in_=rsr_ps[:])
            mrr_ps = pmp.tile([1, E], f32, space="PSUM", tag="m1")
            for ec in range(2):
                nc.tensor.matmul(out=mrr_ps[0:1, ec * P:(ec + 1) * P],
                                 lhsT=murs_col[:, ec:ec + 1], rhs=ident[:],
                                 start=True, stop=True)
            murs_row = pc.tile([1, E], f32)
            nc.vector.tensor_copy(out=murs_row[:], in_=mrr_ps[:])

            sab_ps = pb.tile([P, E], f32, space="PSUM", tag="bc")
            nc.tensor.matmul(out=sab_ps[:], lhsT=sqh_row[0:1, :],
                             rhs=rs_row[0:1, :], start=True, stop=True)
            SA_b = pc.tile([P, E], f32)
            nc.vector.tensor_copy(out=SA_b[:], in_=sab_ps[:])
            smu_ps = pb.tile([P, E], f32, space="PSUM", tag="bc")
            nc.tensor.matmul(out=smu_ps[:], lhsT=sqh_row[0:1, :],
                             rhs=murs_row[0:1, :], start=True, stop=True)
            SAMu_b = pc.tile([P, E], f32)
            nc.vector.tensor_copy(out=SAMu_b[:], in_=smu_ps[:])

            # ---------- phase B (bf16, chunked over tile groups) ----------
            # pos = K0' + sum_e (sa*(|pm| - mu))^2 - sum_e (sa*ps)^2
            #       + sum_e ln ps
            accA = pc.tile([P, T], f32)
            accC = pc.tile([P, T], f32)
            CHG = 3
            for c0 in range(0, T, CHG):
                g = min(CHG, T - c0)
                SA3 = _bc3(SA_b[:], g)
                SAMu3 = _bc3(SAMu_b[:], g)
                pmc = g3[:, c0:c0 + g, 0:E]
                psc = g3[:, c0:c0 + g, E:2 * E]
                tb = pw.tile([P, CHG * E], f32, tag="chA")
                tb3 = tb[:, 0:g * E].rearrange("p (t c) -> p t c", t=g)
                nc.vector.tensor_tensor(out=tb3, in0=pmc, in1=SA3,
                                        op=ALU.mult)
                nc.scalar.activation(out=tb[:, 0:g * E], in_=tb[:, 0:g * E],
                                     func=AF.Abs)
                nc.vector.tensor_tensor(out=tb3, in0=tb3, in1=SAMu3,
                                        op=ALU.subtract)
                nc.scalar.activation(out=tb[:, 0:g * E], in_=tb[:, 0:g * E],
                                     func=AF.Square)
                nc.vector.reduce_sum(out=accA[:, c0:c0 + g], in_=tb3,
                                     axis=mybir.AxisListType.X)
                wbt = pw.tile([P, CHG * E], f32, tag="chB")
                wb3 = wbt[:, 0:g * E].rearrange("p (t c) -> p t c", t=g)
                nc.vector.tensor_tensor(out=wb3, in0=psc, in1=SA3,
                                        op=ALU.mult)
                nc.scalar.activation(out=wbt[:, 0:g * E], in_=wbt[:, 0:g * E],
                                     func=AF.Square)
                nc.vector.reduce_sum(out=accC[:, c0:c0 + g], in_=wb3,
                                     axis=mybir.AxisListType.X)
            pos_all = pc.tile([P, T], f32)
            nc.vector.tensor_tensor(out=pos_all[:], in0=accA[:], in1=accC[:],
                                    op=ALU.subtract)
            nc.vector.tensor_tensor(out=pos_all[:], in0=pos_all[:],
                                    in1=accD[:], op=ALU.add)
            k0b = _bc3(k0col[:], T)  # [P, T, 1] broadcast
            posv = pos_all[:].rearrange("p (t o) -> p t o", t=T)
            nc.vector.tensor_tensor(out=posv, in0=posv, in1=k0b, op=ALU.add)
            nc.vector.tensor_tensor(out=pos_all[:], in0=pos_all[:],
                                    in1=wm_t[:], op=ALU.mult)
            nc.vector.tensor_tensor(out=pos_all[:], in0=pos_all[:],
                                    in1=am_t[:], op=ALU.add)

            # ---------- neg_kl rows ----------
            nacc1 = pc.tile([NEG, 1], f32)
            nacc2 = pc.tile([NEG, 1], f32)
            n1 = pc.tile([NEG, E], f32)
            nc.vector.tensor_tensor(out=n1[:], in0=pmn_t[:], in1=SA_b[0:NEG, :],
                                    op=ALU.mult)
            nc.scalar.activation(out=n1[:], in_=n1[:], func=AF.Abs)
            nc.vector.tensor_tensor(out=n1[:], in0=n1[:], in1=SAMu_b[0:NEG, :],
                                    op=ALU.subtract)
            nc.scalar.activation(out=n1[:], in_=n1[:], func=AF.Square,
                                 accum_out=nacc1[:])
            n2 = pc.tile([NEG, E], f32)
            nc.vector.tensor_tensor(out=n2[:], in0=psn_t[:], in1=SA_b[0:NEG, :],
                                    op=ALU.mult)
            nc.scalar.activation(out=n2[:], in_=n2[:], func=AF.Square,
                                 accum_out=nacc2[:])
            nk = pc.tile([NEG, 1], f32)
            nc.vector.tensor_tensor(out=nk[:], in0=nacc1[:], in1=nacc2[:],
                                    op=ALU.subtract)
            nc.vector.tensor_tensor(out=nk[:], in0=nk[:], in1=nacc3[:],
                                    op=ALU.add)
            nc.vector.tensor_tensor(out=nk[:], in0=nk[:], in1=k0col[0:NEG, :],
                                    op=ALU.add)
            bneg = pc.tile([NEG, 1], f32)
            nc.vector.tensor_scalar(out=bneg[:], in0=nk[:], scalar1=-1.0,
                                    scalar2=1.0, op0=ALU.mult, op1=ALU.add)
            b_ps = pmp.tile([1, NEG], f32, space="PSUM", tag="m1")
            nc.tensor.transpose(out=b_ps[:], in_=bneg[:],
                                identity=ident[0:NEG, 0:NEG])
            b_row = pc.tile([1, NEG], f32)
            nc.vector.tensor_copy(out=b_row[:], in_=b_ps[:])
            bb_ps = pb.tile([P, NEG], f32, space="PSUM", tag="bc")
            nc.tensor.matmul(out=bb_ps[:], lhsT=ones_row[0:1, :],
                             rhs=b_row[0:1, :], start=True, stop=True)
            B_b = pc.tile([P, NEG], f32)
            nc.vector.tensor_copy(out=B_b[:], in_=bb_ps[:])

            # ---------- hinge: relu(pos_c + b_n) over all (n, c) pairs ---
            hb = pc.tile([P, NEG * T], f32)
            hb3 = hb[:].rearrange("p (n t) -> p n t", n=NEG)
            pos_ap = pos_all[:]
            pos_bc = bass.AP(pos_ap.tensor, pos_ap.offset,
                             [pos_ap.ap[0], [0, NEG], pos_ap.ap[1]])
            bb_ap = B_b[:]
            bb_bc = bass.AP(bb_ap.tensor, bb_ap.offset,
                            [bb_ap.ap[0], bb_ap.ap[1], [0, T]])
            nc.vector.tensor_tensor(out=hb3, in0=pos_bc, in1=bb_bc,
                                    op=ALU.add)
            nc.scalar.activation(out=hb[:], in_=hb[:], func=AF.Relu)
            hsumcol = pc.tile([P, 1], f32)
            nc.vector.reduce_sum(out=hsumcol[:], in_=hb3,
                                 axis=mybir.AxisListType.XY)
            lik_ps = pmp.tile([1, 1], f32, space="PSUM", tag="m1")
            nc.tensor.matmul(out=lik_ps[:], lhsT=hsumcol[:], rhs=ones_col[:],
                             start=True, stop=True)
            lik = pc.tile([1, 1], f32)
            nc.scalar.copy(out=lik[:], in_=lik_ps[:])

            # ---------- kl_main in columns ----------
            # 0.5*( sum_e [2 ln psx - ln sig + sig/varp + (pmx-mu)^2/varp]
            #       - E )
            sqp = pc.tile([P, 2], f32)
            nc.scalar.activation(out=sqp[:], in_=psxc_t[:], func=AF.Square)
            rvp = pc.tile([P, 2], f32)
            nc.vector.reciprocal(out=rvp[:], in_=sqp[:])
            kt = pc.tile([P, 2], f32)
            nc.scalar.activation(out=kt[:], in_=psxc_t[:], func=AF.Ln)
            nc.vector.tensor_scalar(out=kt[:], in0=kt[:], scalar1=2.0,
                                    scalar2=None, op0=ALU.mult)
            nc.vector.tensor_tensor(out=kt[:], in0=kt[:], in1=lnsg_col[:],
                                    op=ALU.subtract)
            t1c = pc.tile([P, 2], f32)
            nc.vector.tensor_tensor(out=t1c[:], in0=sig_col[:], in1=rvp[:],
                                    op=ALU.mult)
            nc.vector.tensor_tensor(out=kt[:], in0=kt[:], in1=t1c[:],
                                    op=ALU.add)
            dc = pc.tile([P, 2], f32)
            nc.vector.tensor_tensor(out=dc[:], in0=pmxc_t[:], in1=mu_col[:],
                                    op=ALU.subtract)
            nc.vector.tensor_tensor(out=dc[:], in0=dc[:], in1=dc[:],
                                    op=ALU.mult)
            nc.vector.tensor_tensor(out=dc[:], in0=dc[:], in1=rvp[:],
                                    op=ALU.mult)
            nc.vector.tensor_tensor(out=kt[:], in0=kt[:], in1=dc[:],
                                    op=ALU.add)
            ktr = pc.tile([P, 1], f32)
            nc.vector.reduce_sum(out=ktr[:], in_=kt[:],
                                 axis=mybir.AxisListType.X)
            kl_ps = pmp.tile([1, 1], f32, space="PSUM", tag="m1")
            nc.tensor.matmul(out=kl_ps[:], lhsT=ktr[:], rhs=ones_col[:],
                             start=True, stop=True)
            klm = pc.tile([1, 1], f32)
            nc.scalar.activation(out=klm[:], in_=kl_ps[:], func=AF.Identity,
                                 scale=0.5, bias=cn128[0:1, :])

            # ---------- output row ----------
            orow = pc.tile([1, 8], f32)
            nc.vector.memset(orow[:], 0.0)
            nc.vector.tensor_tensor(out=orow[:, 0:1], in0=lik[:], in1=klm[:],
                                    op=ALU.add)
            nc.vector.tensor_copy(out=orow[:, 1:2], in_=klm[:])
            nc.vector.tensor_copy(out=orow[:, 2:3], in_=lik[:])
            hdr = pc.tile([P, 1], f32)
            nc.vector.reduce_sum(out=hdr[:], in_=ht[:, 2:4],
                                 axis=mybir.AxisListType.X)
            hd_ps = pmp.tile([1, 1], f32, space="PSUM", tag="m1")
            nc.tensor.matmul(out=hd_ps[:], lhsT=hdr[:], rhs=ones_col[:],
                             start=True, stop=True)
            nc.vector.tensor_copy(out=orow[:, 3:4], in_=hd_ps[:])
            sdr = pc.tile([P, 1], f32)
            nc.vector.reduce_sum(out=sdr[:], in_=sig_col[:],
                                 axis=mybir.AxisListType.X)
            sd_ps = pmp.tile([1, 1], f32, space="PSUM", tag="m1")
            nc.tensor.matmul(out=sd_ps[:], lhsT=sdr[:], rhs=ones_col[:],
                             start=True, stop=True)
            nc.vector.tensor_copy(out=orow[:, 4:5], in_=sd_ps[:])
            mdr = pc.tile([P, 1], f32)
            nc.vector.reduce_sum(out=mdr[:], in_=mu_col[:],
                                 axis=mybir.AxisListType.X)
            md_ps = pmp.tile([1, 1], f32, space="PSUM", tag="m1")
            nc.tensor.matmul(out=md_ps[:], lhsT=mdr[:], rhs=ones_col[:],
                             start=True, stop=True)
            nc.vector.tensor_copy(out=orow[:, 5:6], in_=md_ps[:])
            nc.sync.dma_start(out=out[:, :], in_=orow[:])
            if dbg:
                nc.sync.dma_start(out=d_sig[:, 0:2], in_=sig_col[:])
                nc.sync.dma_start(out=d_sig[:, 2:4], in_=rs_col[:])
                nc.sync.dma_start(out=d_sig[:, 4:6], in_=mu_col[:])
                nc.sync.dma_start(out=d_sig[:, 6:8], in_=z_col[:])
                nc.sync.dma_start(out=d_pos[:, 0:T], in_=pos_all[:])
    nc.compile()
    return nc


def _prep_inputs(x, context, negative_samples, emb, M_w, M_b, U_w, U_b,
                 W_w, W_b, prior_mus, prior_sigmas):
    x = np.asarray(x).reshape(-1)
    ctx = np.asarray(context).reshape(-1).astype(np.int64)
    neg = np.asarray(negative_samples).reshape(-1).astype(np.int64)
    emb = np.ascontiguousarray(np.asarray(emb, dtype=np.float32))
    prior_mus = np.ascontiguousarray(np.asarray(prior_mus, dtype=np.float32))
    prior_sigmas = np.ascontiguousarray(np.asarray(prior_sigmas,
                                                   dtype=np.float32))

    owner = ctx // SH
    local = (ctx % SH).astype(np.int32)
    counts = np.bincount(owner, minlength=M)
    T = max(1, int(np.ceil(counts.max() / P)))

    packB = np.concatenate([prior_mus, prior_sigmas], axis=1)  # [V, 512]

    import ml_dtypes
    bfl = ml_dtypes.bfloat16
    mwt = np.ascontiguousarray(np.asarray(M_w, np.float32).T.astype(bfl))
    uwwt = np.ascontiguousarray(np.concatenate(
        [np.asarray(U_w, np.float32).T, np.asarray(W_w, np.float32).T],
        axis=1))  # [512, 512] = [U^T | W^T]
    mb = np.asarray(M_b, np.float32).reshape(1, E).astype(bfl)
    ubwb = np.concatenate([np.asarray(U_b, np.float32).reshape(1, E),
                           np.asarray(W_b, np.float32).reshape(1, E)], axis=1)
    xi = int(x[0])
    embx = np.ascontiguousarray(emb[xi].reshape(2, P).T.astype(bfl))
    pmxc = np.ascontiguousarray(prior_mus[xi].reshape(2, P).T)
    psxc = np.ascontiguousarray(prior_sigmas[xi].reshape(2, P).T)
    pmn = np.ascontiguousarray(prior_mus[neg])
    psn = np.ascontiguousarray(prior_sigmas[neg])
    idf = np.eye(P, dtype=np.float32)
    idb = np.eye(P, dtype=np.float32).astype(bfl)

    in_maps = []
    for k in range(M):
        sel = np.nonzero(owner == k)[0]
        nk_ = len(sel)
        idxk = np.zeros(T * P, np.int32)
        idxk[:nk_] = local[sel]
        wmk = np.zeros(T * P, np.float32)
        wmk[:nk_] = 1.0
        amk = np.full(T * P, -1e30, np.float32)
        amk[:nk_] = 0.0
        wm_pt = np.ascontiguousarray(wmk.reshape(T, P).T)
        in_maps.append({
            "packA": emb[k * SH:(k + 1) * SH],
            "packB": packB[k * SH:(k + 1) * SH],
            "idx": np.ascontiguousarray(idxk.reshape(T, P).T),
            "wm": wm_pt,
            "wmb": wm_pt.astype(bfl),
            "am": np.ascontiguousarray(amk.reshape(T, P).T),
            "mwt": mwt, "mb": mb, "uwwt": uwwt, "ubwb": ubwb,
            "embx": embx, "pmxc": pmxc, "psxc": psxc, "pmn": pmn, "psn": psn,
            "idf": idf, "idb": idb,
        })
    return T, in_maps


def run_on_device(T, in_maps, trace=False):
    nc = _BUILD_CACHE.get(T)
    if nc is None:
        nc = _build(T)
        _BUILD_CACHE[T] = nc
    return bass_utils.run_bass_kernel_spmd(
        nc, in_maps, core_ids=list(range(M)), trace=trace)


def kernel(**inputs) -> np.ndarray:
    T, in_maps = _prep_inputs(**inputs)
    res = run_on_device(T, in_maps)
    outs = [res.results[k]["out"][0] for k in range(M)]
    lik = np.float32(0.0)
    for k in range(M):
        lik = np.float32(lik + np.float32(outs[k][2]))
    total = np.float32(np.float32(outs[0][1]) + lik)
    return np.array([total], dtype=np.float32)


if __name__ == "__main__":
    d = np.load("/tmp/ref_inputs.npz")
    inp = {k: d[k] for k in d.files}
    out = kernel(**inp)
    print("kernel output:", out)
